# revision 13
# baseline (speedup 1.0000x reference)
"""TRN2 Bass kernel for nn_CamAwareSCLHead: cam-aware supervised contrastive loss.

Strategy (data-parallel over 8 NeuronCores, 1024 of 8192 logit rows each):
  host: sort samples by label so same-label positives form a narrow diagonal
        band; rotate each core's column space so its rows sit at cols [0,1024).
  device per row-stripe of 128 rows:
    PE   : fp16 matmul f_local @ f_full.T in [128,2048] chunks (fp32 accum),
           then identity-matmuls add -60000 on the diagonal and on all
           positive pairs (so the dense exp-sum covers negatives only).
    DVE  : per-chunk row max (negated).
    ACT  : exp(lg - chunkmax) with fused row-sum accumulation.
    band : raw logits of the +-24 diagonal band (positives live there) are
           copied out, bounced through DRAM to extract the per-row diagonal
           window, and all the log/mask math runs on tiny [128,768] tensors.
  The per-positive loss terms replicate the fp32 reference exactly, including
  its underflow behaviour (the fp32 reference of this problem is -inf).
"""
import sys

if '/opt/trn_rl_repo' not in sys.path:
    sys.path.insert(0, '/opt/trn_rl_repo')

import numpy as np

N = 4096
D = 256
N2 = 2 * N
TEMP = 0.1
NCORE = 8
RPC = N2 // NCORE          # rows per core
NSTR = RPC // 128          # row stripes per core
W = 24                     # half window of the diagonal band
KW = 2 * W                 # gathered slots per band side
BANDW = 128 + KW           # band buffer cols per side
BW2 = 2 * BANDW
BPAD = BANDW * 2 + 2       # dram row stride 354 => diagonal becomes p*354+k
CHUNK = 2048
NCH = N2 // CHUNK
KILL = np.float32(-60000.0)

_CACHE = {}
DEBUG_TAPS = False


# --------------------------------------------------------------------------
# host-side prep: sorting, rotation, window weights, kill masks
# --------------------------------------------------------------------------

def _host_prep(features, label, camid):
    f = np.concatenate([features[:, 0], features[:, 1]], axis=0).astype(np.float32)
    f = f * np.float32(np.sqrt(1.0 / TEMP))   # bake 1/TEMP into the matmul
    label = np.asarray(label).astype(np.int64)
    camid = np.asarray(camid).astype(np.int64)
    order = np.argsort(label, kind='stable')
    label_s = label[order]
    cam_s = camid[order]
    f_s = np.concatenate([f[:N][order], f[N:][order]], axis=0)

    # label block [start, end) per sorted sample
    change = np.flatnonzero(np.diff(label_s)) + 1
    bounds = np.concatenate([[0], change, [N]])
    blk_start = np.zeros(N, np.int64)
    blk_end = np.zeros(N, np.int64)
    for i in range(len(bounds) - 1):
        blk_start[bounds[i]:bounds[i + 1]] = bounds[i]
        blk_end[bounds[i]:bounds[i + 1]] = bounds[i + 1]
    assert (blk_end - blk_start).max() <= W, "label block exceeds band window"

    per_core = []
    for c in range(NCORE):
        rot = np.roll(f_s, -RPC * c, axis=0)
        fT = np.ascontiguousarray(rot.T).astype(np.float16)      # [D, N2]
        fT = fT.reshape(2, 128, N2)

        Wid = np.zeros((RPC, 2 * KW), np.float32)
        Wcam = np.zeros((RPC, 2 * KW), np.float32)
        Kpos = np.zeros((RPC, BW2), np.float32)
        p_arr = np.arange(RPC)
        g_arr = RPC * c + p_arr
        sig = g_arr % N
        for p in range(RPC):
            a, b = blk_start[sig[p]], blk_end[sig[p]]
            prow = p % 128
            for j in range(a, b):
                for half in (0, 1):
                    gcol = j + half * N
                    q = (gcol - RPC * c) % N2
                    dlt = (q - p + N2 // 2) % N2 - N2 // 2
                    if -W <= dlt < W:
                        k = dlt + W
                        u = prow + dlt + W
                    else:
                        d2 = dlt - N if dlt > 0 else dlt + N
                        assert -W <= d2 < W
                        k = KW + d2 + W
                        u = BANDW + prow + d2 + W
                    if gcol == (RPC * c + p):
                        continue                       # diag handled separately
                    Kpos[p, u] = KILL
                    Wid[p, k] = 1.0
                    if cam_s[j] == cam_s[sig[p]]:
                        Wcam[p, k] = 1.0

        def fold(a, width):        # [RPC, width] -> [128, NSTR*width]
            return np.ascontiguousarray(
                a.reshape(NSTR, 128, width).transpose(1, 0, 2).reshape(128, NSTR * width))

        rnid = (1.0 / Wid.sum(axis=1)).astype(np.float32)
        rncam = (1.0 / Wcam.sum(axis=1)).astype(np.float32)
        per_core.append({
            "ft": fT,
            "wid": fold(Wid, 2 * KW),
            "wcam": fold(Wcam, 2 * KW),
            "wdiff": fold(Wid - Wcam, 2 * KW),
            "kpos": fold(Kpos, BW2).astype(np.float16),
            "rnid": fold(rnid[:, None], 1),
            "rncam": fold(rncam[:, None], 1),
        })
    ident = np.eye(128, dtype=np.float16)
    kdiag = (np.float32(KILL) * np.eye(128)).astype(np.float16)
    for pc in per_core:
        pc["ident"] = ident
        pc["kdiag"] = kdiag
    return per_core, order


# --------------------------------------------------------------------------
# band piece geometry (static, same for all cores thanks to rotation)
# --------------------------------------------------------------------------

def _band_pieces(s, side):
    """Return [(chunk, off_in_chunk, width, buf_off)] for band `side` of stripe s.
    Splits at the N2 wrap; pieces are also split at 512 psum-bank boundaries
    by the caller when used for kill matmuls."""
    base = 128 * s - W + (N if side else 0)
    pieces = []
    u = 0
    while u < BANDW:
        col = (base + u) % N2
        # width until wrap or end
        wdt = min(BANDW - u, N2 - col)
        # split at chunk boundary
        ch = col // CHUNK
        wdt = min(wdt, (ch + 1) * CHUNK - col)
        pieces.append((ch, col - ch * CHUNK, wdt, u))
        u += wdt
    return pieces


def _split_bank(pieces):
    out = []
    for ch, off, wdt, u in pieces:
        while wdt > 0:
            w1 = min(wdt, 512 - (off % 512))
            out.append((ch, off, w1, u))
            off += w1; u += w1; wdt -= w1
    return out


# --------------------------------------------------------------------------
# bass kernel
# --------------------------------------------------------------------------

def _build():
    if "nc" in _CACHE:
        return _CACHE["nc"]
    import concourse.bass as bass
    import concourse.bacc as bacc
    import concourse.tile as tile
    from concourse import mybir

    f32 = mybir.dt.float32
    f16 = mybir.dt.float16

    nc = bacc.Bacc("TRN2", target_bir_lowering=False, debug=False,
                   num_devices=NCORE)

    ft_d = nc.dram_tensor("ft", [2, 128, N2], f16, kind="ExternalInput")
    wid_d = nc.dram_tensor("wid", [128, NSTR * 2 * KW], f32, kind="ExternalInput")
    wcam_d = nc.dram_tensor("wcam", [128, NSTR * 2 * KW], f32, kind="ExternalInput")
    wdiff_d = nc.dram_tensor("wdiff", [128, NSTR * 2 * KW], f32, kind="ExternalInput")
    kpos_d = nc.dram_tensor("kpos", [128, NSTR * BW2], f16, kind="ExternalInput")
    rnid_d = nc.dram_tensor("rnid", [128, NSTR], f32, kind="ExternalInput")
    rncam_d = nc.dram_tensor("rncam", [128, NSTR], f32, kind="ExternalInput")
    ident_d = nc.dram_tensor("ident", [128, 128], f16, kind="ExternalInput")
    kdiag_d = nc.dram_tensor("kdiag", [128, 128], f16, kind="ExternalInput")
    out_d = nc.dram_tensor("out", [128, 1], f32, kind="ExternalOutput")
    if DEBUG_TAPS:
        dbg_g = nc.dram_tensor("dbg_g", [128, NSTR * 2 * KW], f32, kind="ExternalOutput")
        dbg_negq = nc.dram_tensor("dbg_negq", [128, NSTR * NCH], f32, kind="ExternalOutput")
        dbg_ssum = nc.dram_tensor("dbg_ssum", [128, NSTR * NCH], f32, kind="ExternalOutput")
        dbg_band0 = nc.dram_tensor("dbg_band0", [128, BW2], f32, kind="ExternalOutput")
        dbg_S = nc.dram_tensor("dbg_S", [128, NSTR], f32, kind="ExternalOutput")
        dbg_eb = nc.dram_tensor("dbg_eb", [128, NSTR * 2 * KW], f32, kind="ExternalOutput")
        dbg_gm = nc.dram_tensor("dbg_gm", [128, NSTR * 2 * KW], f32, kind="ExternalOutput")

    GW = 2 * KW  # 96 gathered cols per stripe

    with tile.TileContext(nc) as tc:
        with (
            tc.tile_pool(name="persist", bufs=1) as pp,
            tc.tile_pool(name="bands", bufs=2) as bandp,
            tc.tile_pool(name="trash", bufs=2) as trashp,
            tc.tile_pool(name="psum", bufs=2, space="PSUM") as psp,
            tc.tile_pool(name="dramband", bufs=2, space="DRAM") as dbp,
            tc.tile_pool(name="endp", bufs=2) as ep,
        ):
            # ---- persistent inputs -> SBUF
            ft_sb = [pp.tile([128, N2], f16, tag=f"ft{k}", name=f"ftsb{k}") for k in range(2)]
            for k in range(2):
                nc.sync.dma_start(ft_sb[k][:], ft_d[k])
            wid_sb = pp.tile([128, NSTR * GW], f32, tag="wid")
            wcam_sb = pp.tile([128, NSTR * GW], f32, tag="wcam")
            wdiff_sb = pp.tile([128, NSTR * GW], f32, tag="wdiff")
            kpos_sb = pp.tile([128, NSTR * BW2], f16, tag="kpos")
            rnid_sb = pp.tile([128, NSTR], f32, tag="rnid")
            rncam_sb = pp.tile([128, NSTR], f32, tag="rncam")
            ident_sb = pp.tile([128, 128], f16, tag="ident")
            kdiag_sb = pp.tile([128, 128], f16, tag="kdiag")
            for sb, dr in ((wid_sb, wid_d), (wcam_sb, wcam_d), (wdiff_sb, wdiff_d),
                           (kpos_sb, kpos_d), (rnid_sb, rnid_d), (rncam_sb, rncam_d),
                           (ident_sb, ident_d), (kdiag_sb, kdiag_d)):
                nc.sync.dma_start(sb[:], dr.ap())

            negq_sb = pp.tile([128, NSTR * NCH], f32, tag="negq")
            ssum_sb = pp.tile([128, NSTR * NCH], f32, tag="ssum")
            g_sb = pp.tile([128, NSTR * GW], f32, tag="g")

            # ---- main loop over stripes / chunks
            for s in range(NSTR):
                lhs = [ft_sb[k][:, 128 * s:128 * (s + 1)] for k in range(2)]
                psum_chunks = []
                for ch in range(NCH):
                    pt = psp.tile([128, CHUNK], f32, tag="ck", name=f"ck{ch}")
                    psum_chunks.append(pt)
                    for k in range(2):
                        for t in range(CHUNK // 512):
                            col0 = ch * CHUNK + t * 512
                            nc.tensor.matmul(
                                pt[:, t * 512:(t + 1) * 512],
                                lhs[k],
                                ft_sb[k][:, col0:col0 + 512],
                                start=(k == 0), stop=False,
                                skip_group_check=True,
                            )
                # diag kill: cols [128s, 128s+128) always inside chunk 0
                dk_ch = (128 * s) // CHUNK
                dk_off = 128 * s - dk_ch * CHUNK
                nc.tensor.matmul(
                    psum_chunks[dk_ch][:, dk_off:dk_off + 128],
                    kdiag_sb[:], ident_sb[:],
                    start=False, stop=False, skip_group_check=True)

                # band copy (raw logits; diag already dead)
                band_sb = bandp.tile([128, BW2], f32, tag="band")
                for side in (0, 1):
                    for ch, off, wdt, u in _band_pieces(s, side):
                        nc.scalar.copy(
                            band_sb[:, side * BANDW + u: side * BANDW + u + wdt],
                            psum_chunks[ch][:, off:off + wdt])

                # positive kill matmuls
                for side in (0, 1):
                    for ch, off, wdt, u in _split_bank(_band_pieces(s, side)):
                        nc.tensor.matmul(
                            psum_chunks[ch][:, off:off + wdt],
                            ident_sb[:],
                            kpos_sb[:, s * BW2 + side * BANDW + u:
                                       s * BW2 + side * BANDW + u + wdt],
                            start=False, stop=(ch == NCH - 1 and side == 1),
                            skip_group_check=True)

                # chunk max (negated) then exp+accum over killed logits
                for ch in range(NCH):
                    slot = s * NCH + ch
                    nc.vector.tensor_reduce(
                        negq_sb[:, slot:slot + 1], psum_chunks[ch][:],
                        axis=mybir.AxisListType.X, op=mybir.AluOpType.max,
                        negate=True)
                    trash = trashp.tile([128, CHUNK], f32, tag="trash")
                    nc.scalar.activation(
                        trash[:], psum_chunks[ch][:],
                        mybir.ActivationFunctionType.Exp,
                        bias=negq_sb[:, slot:slot + 1],
                        accum_out=ssum_sb[:, slot:slot + 1])

                if DEBUG_TAPS and s == 0:
                    nc.sync.dma_start(dbg_band0.ap(), band_sb[:])

                # band -> DRAM -> diagonal gather.  dram rows have stride
                # BW2+2=354; the per-row diagonal window [p, p+k] linearizes
                # to flat[p*355 + k], read back via a stride-355 view.
                dband = dbp.tile([128 * (BW2 + 3)], f32, tag="dband")
                flat = dband[:]
                wview = flat[0:128 * (BW2 + 2)].rearrange(
                    "(p u) -> p u", u=BW2 + 2)
                nc.sync.dma_start(wview[:, 0:BW2], band_sb[:])
                rview = flat[0:128 * (BW2 + 3)].rearrange(
                    "(p x) -> p x", x=BW2 + 3)
                nc.sync.dma_start(g_sb[:, s * GW: s * GW + KW], rview[:, 0:KW])
                nc.sync.dma_start(g_sb[:, s * GW + KW: s * GW + 2 * KW],
                                  rview[:, BANDW:BANDW + KW])

            if DEBUG_TAPS:
                nc.sync.dma_start(dbg_g.ap(), g_sb[:])
                nc.sync.dma_start(dbg_negq.ap(), negq_sb[:])
                nc.sync.dma_start(dbg_ssum.ap(), ssum_sb[:])

            # ---- end phase (batched over all stripes) --------------------
            negq3 = negq_sb[:].rearrange("p (s c) -> p s c", c=NCH)
            g3 = g_sb[:].rearrange("p (s k) -> p s k", k=GW)

            negbm = ep.tile([128, NSTR], f32, tag="negbm")
            nc.vector.tensor_reduce(negbm[:], g3, axis=mybir.AxisListType.X,
                                    op=mybir.AluOpType.max, negate=True)
            negm = ep.tile([128, NSTR], f32, tag="negm")
            nc.vector.tensor_reduce(negm[:], negq3, axis=mybir.AxisListType.X,
                                    op=mybir.AluOpType.min)
            nc.vector.tensor_tensor(negm[:], negm[:], negbm[:],
                                    op=mybir.AluOpType.min)

            def bcast(small, k, tag):
                # [128, NSTR] -> [128, NSTR, k] broadcast AP
                return small[:].unsqueeze(2).to_broadcast((128, NSTR, k))

            # S = sum_c Ssum_c * exp(negm - negq_c)
            efac = ep.tile([128, NSTR * NCH], f32, tag="efac")
            efac3 = efac[:].rearrange("p (s c) -> p s c", c=NCH)
            nc.vector.tensor_tensor(efac3, bcast(negm, NCH, "negmb"), negq3,
                                    op=mybir.AluOpType.subtract)
            nc.scalar.activation(efac[:], efac[:],
                                 mybir.ActivationFunctionType.Exp)
            nc.vector.tensor_tensor(efac[:], efac[:], ssum_sb[:],
                                    op=mybir.AluOpType.mult)
            S = ep.tile([128, NSTR], f32, tag="S")
            nc.vector.tensor_reduce(S[:], efac3, axis=mybir.AxisListType.X,
                                    op=mybir.AluOpType.add)

            if DEBUG_TAPS:
                nc.sync.dma_start(dbg_S.ap(), S[:])

            # gm = g + negm ; eb = exp(gm)
            gm = ep.tile([128, NSTR * GW], f32, tag="gm")
            gm3 = gm[:].rearrange("p (s k) -> p s k", k=GW)
            nc.vector.tensor_tensor(gm3, g3, bcast(negm, GW, "negmb2"),
                                    op=mybir.AluOpType.add)
            eb = ep.tile([128, NSTR * GW], f32, tag="eb")
            nc.scalar.activation(eb[:], gm[:], mybir.ActivationFunctionType.Exp)

            if DEBUG_TAPS:
                nc.sync.dma_start(dbg_eb.ap(), eb[:])
                nc.sync.dma_start(dbg_gm.ap(), gm[:])

            # ns_cam = S + sum (wid-wcam)*eb ; ns_id = S
            tmp = ep.tile([128, NSTR * GW], f32, tag="tmp")
            tmp3 = tmp[:].rearrange("p (s k) -> p s k", k=GW)
            nc.vector.tensor_tensor(tmp[:], eb[:], wdiff_sb[:],
                                    op=mybir.AluOpType.mult)
            nscam = ep.tile([128, NSTR], f32, tag="nscam")
            nc.vector.tensor_reduce(nscam[:], tmp3, axis=mybir.AxisListType.X,
                                    op=mybir.AluOpType.add)
            nc.vector.tensor_tensor(nscam[:], nscam[:], S[:],
                                    op=mybir.AluOpType.add)

            rows = ep.tile([128, NSTR], f32, tag="rows")
            first = True
            for wsb, nssb, rnsb in ((wid_sb, S, rnid_sb),
                                    (wcam_sb, nscam, rncam_sb)):
                # t = ((eb - W) + 1) + ns  — order preserves tiny ns exactly
                nc.vector.tensor_tensor(tmp[:], eb[:], wsb[:],
                                        op=mybir.AluOpType.subtract)
                nc.vector.scalar_tensor_tensor(
                    tmp3, tmp3, 1.0, bcast(nssb, GW, "nsb"),
                    op0=mybir.AluOpType.add, op1=mybir.AluOpType.add)
                nc.scalar.activation(tmp[:], tmp[:],
                                     mybir.ActivationFunctionType.Ln)
                nc.vector.tensor_tensor(tmp[:], tmp[:], gm[:],
                                        op=mybir.AluOpType.subtract)
                nc.vector.tensor_tensor(tmp[:], tmp[:], wsb[:],
                                        op=mybir.AluOpType.mult)
                L = ep.tile([128, NSTR], f32, tag="L")
                nc.vector.tensor_reduce(L[:], tmp3, axis=mybir.AxisListType.X,
                                        op=mybir.AluOpType.add)
                nc.vector.tensor_tensor(L[:], L[:], rnsb[:],
                                        op=mybir.AluOpType.mult)
                if first:
                    nc.vector.tensor_copy(rows[:], L[:])
                    first = False
                else:
                    nc.vector.scalar_tensor_tensor(
                        rows[:], L[:], 0.5, rows[:],
                        op0=mybir.AluOpType.mult, op1=mybir.AluOpType.add)

            rowsum = ep.tile([128, 1], f32, tag="rowsum")
            nc.vector.tensor_reduce(rowsum[:], rows[:],
                                    axis=mybir.AxisListType.X,
                                    op=mybir.AluOpType.add)
            nc.sync.dma_start(out_d.ap(), rowsum[:])

    nc.compile()
    _CACHE["nc"] = nc
    return nc


# --------------------------------------------------------------------------
# public entry point
# --------------------------------------------------------------------------

def kernel(features, label, camid):
    from concourse.bass_utils import run_bass_kernel_spmd

    features = np.asarray(features, np.float32)
    label_in = np.asarray(label)
    camid_in = np.asarray(camid)

    per_core, _ = _host_prep(features, label_in.astype(np.int64),
                             camid_in.astype(np.int64))
    nc = _build()
    in_maps = [{k: np.ascontiguousarray(v) for k, v in pc.items()}
               for pc in per_core]
    res = run_bass_kernel_spmd(nc, in_maps, core_ids=list(range(NCORE)))
    total = np.float32(0.0)
    outs = []
    for r in res.results:
        o = r["out"].astype(np.float32)
        outs.append(o)
        total += o.sum(dtype=np.float32)
    _CACHE["outs"] = outs
    loss = total / np.float32(N2)
    return np.asarray(loss, dtype=np.float32)


# revision 15
# speedup vs baseline: 1.1510x; 1.1510x over previous
"""TRN2 Bass kernel for nn_CamAwareSCLHead: cam-aware supervised contrastive loss.

Strategy (data-parallel over 8 NeuronCores, 1024 of 8192 logit rows each):
  host: sort samples by label so same-label positives form a narrow diagonal
        band; rotate each core's column space so its rows sit at cols [0,1024).
  device per row-stripe of 128 rows:
    PE   : fp16 matmul f_local @ f_full.T in [128,2048] chunks (fp32 accum),
           then identity-matmuls add -60000 on the diagonal and on all
           positive pairs (so the dense exp-sum covers negatives only).
    DVE  : per-chunk row max (negated).
    ACT  : exp(lg - chunkmax) with fused row-sum accumulation.
    band : raw logits of the +-24 diagonal band (positives live there) are
           copied out, bounced through DRAM to extract the per-row diagonal
           window, and all the log/mask math runs on tiny [128,768] tensors.
  The per-positive loss terms replicate the fp32 reference exactly, including
  its underflow behaviour (the fp32 reference of this problem is -inf).
"""
import sys

if '/opt/trn_rl_repo' not in sys.path:
    sys.path.insert(0, '/opt/trn_rl_repo')

import numpy as np

N = 4096
D = 256
N2 = 2 * N
TEMP = 0.1
NCORE = 8
RPC = N2 // NCORE          # rows per core
NSTR = RPC // 128          # row stripes per core
W = 24                     # half window of the diagonal band
KW = 2 * W                 # gathered slots per band side
BANDW = 128 + KW           # band buffer cols per side
BW2 = 2 * BANDW
BPAD = BANDW * 2 + 2       # dram row stride 354 => diagonal becomes p*354+k
CHUNK = 1024
NCH = N2 // CHUNK
KILL = np.float32(-60000.0)

_CACHE = {}
DEBUG_TAPS = False


# --------------------------------------------------------------------------
# host-side prep: sorting, rotation, window weights, kill masks
# --------------------------------------------------------------------------

def _host_prep(features, label, camid):
    f = np.concatenate([features[:, 0], features[:, 1]], axis=0).astype(np.float32)
    f = f * np.float32(np.sqrt(1.0 / TEMP))   # bake 1/TEMP into the matmul
    label = np.asarray(label).astype(np.int64)
    camid = np.asarray(camid).astype(np.int64)
    order = np.argsort(label, kind='stable')
    label_s = label[order]
    cam_s = camid[order]
    f_s = np.concatenate([f[:N][order], f[N:][order]], axis=0)

    # label block [start, end) per sorted sample
    change = np.flatnonzero(np.diff(label_s)) + 1
    bounds = np.concatenate([[0], change, [N]])
    blk_start = np.zeros(N, np.int64)
    blk_end = np.zeros(N, np.int64)
    for i in range(len(bounds) - 1):
        blk_start[bounds[i]:bounds[i + 1]] = bounds[i]
        blk_end[bounds[i]:bounds[i + 1]] = bounds[i + 1]
    assert (blk_end - blk_start).max() <= W, "label block exceeds band window"

    per_core = []
    for c in range(NCORE):
        rot = np.roll(f_s, -RPC * c, axis=0)
        fT = np.ascontiguousarray(rot.T).astype(np.float16)      # [D, N2]
        fT = np.ascontiguousarray(
            fT.reshape(2, 128, NCH, CHUNK).transpose(0, 2, 1, 3))  # [2,NCH,128,CHUNK]

        Wid = np.zeros((RPC, 2 * KW), np.float32)
        Wcam = np.zeros((RPC, 2 * KW), np.float32)
        Kpos = np.zeros((RPC, BW2), np.float32)
        p_arr = np.arange(RPC)
        g_arr = RPC * c + p_arr
        sig = g_arr % N
        for p in range(RPC):
            a, b = blk_start[sig[p]], blk_end[sig[p]]
            prow = p % 128
            for j in range(a, b):
                for half in (0, 1):
                    gcol = j + half * N
                    q = (gcol - RPC * c) % N2
                    dlt = (q - p + N2 // 2) % N2 - N2 // 2
                    if -W <= dlt < W:
                        k = dlt + W
                        u = prow + dlt + W
                    else:
                        d2 = dlt - N if dlt > 0 else dlt + N
                        assert -W <= d2 < W
                        k = KW + d2 + W
                        u = BANDW + prow + d2 + W
                    if gcol == (RPC * c + p):
                        continue                       # diag handled separately
                    Kpos[p, u] = KILL
                    Wid[p, k] = 1.0
                    if cam_s[j] == cam_s[sig[p]]:
                        Wcam[p, k] = 1.0

        def fold(a, width):        # [RPC, width] -> [128, NSTR*width]
            return np.ascontiguousarray(
                a.reshape(NSTR, 128, width).transpose(1, 0, 2).reshape(128, NSTR * width))

        rnid = (1.0 / Wid.sum(axis=1)).astype(np.float32)
        rncam = (1.0 / Wcam.sum(axis=1)).astype(np.float32)
        per_core.append({
            "ft": fT,
            "wid": fold(Wid, 2 * KW),
            "wcam": fold(Wcam, 2 * KW),
            "wdiff": fold(Wid - Wcam, 2 * KW),
            "kpos": fold(Kpos, BW2).astype(np.float16),
            "rnid": fold(rnid[:, None], 1),
            "rncam": fold(rncam[:, None], 1),
        })
    ident = np.eye(128, dtype=np.float16)
    kdiag = (np.float32(KILL) * np.eye(128)).astype(np.float16)
    for pc in per_core:
        pc["ident"] = ident
        pc["kdiag"] = kdiag
    return per_core, order


# --------------------------------------------------------------------------
# band piece geometry (static, same for all cores thanks to rotation)
# --------------------------------------------------------------------------

def _band_pieces(s, side):
    """Return [(chunk, off_in_chunk, width, buf_off)] for band `side` of stripe s.
    Splits at the N2 wrap; pieces are also split at 512 psum-bank boundaries
    by the caller when used for kill matmuls."""
    base = 128 * s - W + (N if side else 0)
    pieces = []
    u = 0
    while u < BANDW:
        col = (base + u) % N2
        # width until wrap or end
        wdt = min(BANDW - u, N2 - col)
        # split at chunk boundary
        ch = col // CHUNK
        wdt = min(wdt, (ch + 1) * CHUNK - col)
        pieces.append((ch, col - ch * CHUNK, wdt, u))
        u += wdt
    return pieces


def _split_bank(pieces):
    out = []
    for ch, off, wdt, u in pieces:
        while wdt > 0:
            w1 = min(wdt, 512 - (off % 512))
            out.append((ch, off, w1, u))
            off += w1; u += w1; wdt -= w1
    return out


# --------------------------------------------------------------------------
# bass kernel
# --------------------------------------------------------------------------

def _build():
    if "nc" in _CACHE:
        return _CACHE["nc"]
    import concourse.bass as bass
    import concourse.bacc as bacc
    import concourse.tile as tile
    from concourse import mybir

    f32 = mybir.dt.float32
    f16 = mybir.dt.float16

    nc = bacc.Bacc("TRN2", target_bir_lowering=False, debug=False,
                   num_devices=NCORE)

    ft_d = nc.dram_tensor("ft", [2, NCH, 128, CHUNK], f16, kind="ExternalInput")
    wid_d = nc.dram_tensor("wid", [128, NSTR * 2 * KW], f32, kind="ExternalInput")
    wcam_d = nc.dram_tensor("wcam", [128, NSTR * 2 * KW], f32, kind="ExternalInput")
    wdiff_d = nc.dram_tensor("wdiff", [128, NSTR * 2 * KW], f32, kind="ExternalInput")
    kpos_d = nc.dram_tensor("kpos", [128, NSTR * BW2], f16, kind="ExternalInput")
    rnid_d = nc.dram_tensor("rnid", [128, NSTR], f32, kind="ExternalInput")
    rncam_d = nc.dram_tensor("rncam", [128, NSTR], f32, kind="ExternalInput")
    ident_d = nc.dram_tensor("ident", [128, 128], f16, kind="ExternalInput")
    kdiag_d = nc.dram_tensor("kdiag", [128, 128], f16, kind="ExternalInput")
    out_d = nc.dram_tensor("out", [128, 1], f32, kind="ExternalOutput")

    GW = 2 * KW  # 96 gathered cols per stripe

    with tile.TileContext(nc) as tc:
        with (
            tc.tile_pool(name="persist", bufs=1) as pp,
            tc.tile_pool(name="bands", bufs=3) as bandp,
            tc.tile_pool(name="trash", bufs=3) as trashp,
            tc.tile_pool(name="psum", bufs=4, space="PSUM") as psp,
            tc.tile_pool(name="dramband", bufs=3, space="DRAM") as dbp,
            tc.tile_pool(name="endp", bufs=2) as ep,
        ):
            # ---- persistent inputs -> SBUF (ft as per-chunk tiles so the
            # first matmuls start as soon as their columns land)
            ft_sb = [[pp.tile([128, CHUNK], f16, tag=f"ft{k}_{ch}",
                              name=f"ftsb{k}_{ch}") for ch in range(NCH)]
                     for k in range(2)]
            for k in range(2):
                for ch in range(NCH):
                    nc.sync.dma_start(ft_sb[k][ch][:], ft_d[k, ch])
            wid_sb = pp.tile([128, NSTR * GW], f32, tag="wid")
            wcam_sb = pp.tile([128, NSTR * GW], f32, tag="wcam")
            wdiff_sb = pp.tile([128, NSTR * GW], f32, tag="wdiff")
            kpos_sb = pp.tile([128, NSTR * BW2], f16, tag="kpos")
            rnid_sb = pp.tile([128, NSTR], f32, tag="rnid")
            rncam_sb = pp.tile([128, NSTR], f32, tag="rncam")
            ident_sb = pp.tile([128, 128], f16, tag="ident")
            kdiag_sb = pp.tile([128, 128], f16, tag="kdiag")
            for sb, dr in ((kpos_sb, kpos_d), (ident_sb, ident_d),
                           (kdiag_sb, kdiag_d), (wid_sb, wid_d),
                           (wcam_sb, wcam_d), (wdiff_sb, wdiff_d),
                           (rnid_sb, rnid_d), (rncam_sb, rncam_d)):
                nc.sync.dma_start(sb[:], dr.ap())

            negq_sb = pp.tile([128, NSTR * NCH], f32, tag="negq")
            ssum_sb = pp.tile([128, NSTR * NCH], f32, tag="ssum")
            g_sb = pp.tile([128, NSTR * GW], f32, tag="g")

            # ---- main loop over stripes / chunks
            for s in range(NSTR):
                psum_chunks = []
                band_sb = bandp.tile([128, BW2], f32, tag="band",
                                     name=f"band{s}")
                # which chunks have band pieces (they need copies+kills before
                # the max/exp pass)
                piece_map = {}
                for side in (0, 1):
                    for pc in _band_pieces(s, side):
                        piece_map.setdefault(pc[0], []).append((side, pc))
                for ch in range(NCH):
                    pt = psp.tile([128, CHUNK], f32, tag="ck", name=f"ck{s}_{ch}")
                    psum_chunks.append(pt)
                    for k in range(2):
                        lhs = ft_sb[k][0][:, 128 * s:128 * (s + 1)]
                        for t in range(CHUNK // 512):
                            nc.tensor.matmul(
                                pt[:, t * 512:(t + 1) * 512],
                                lhs,
                                ft_sb[k][ch][:, t * 512:(t + 1) * 512],
                                start=(k == 0), stop=False,
                                skip_group_check=True,
                            )
                    # diag kill lives in chunk (128*s)//CHUNK
                    if ch == (128 * s) // CHUNK:
                        off = 128 * s - ch * CHUNK
                        nc.tensor.matmul(
                            pt[:, off:off + 128],
                            kdiag_sb[:], ident_sb[:],
                            start=False, stop=False, skip_group_check=True)
                    # band copies (raw, diag dead) then positive-kill matmuls
                    for side, (pch, off, wdt, u) in piece_map.get(ch, []):
                        nc.scalar.copy(
                            band_sb[:, side * BANDW + u: side * BANDW + u + wdt],
                            pt[:, off:off + wdt])
                    for side, piece in piece_map.get(ch, []):
                        for pch, off, wdt, u in _split_bank([piece]):
                            nc.tensor.matmul(
                                pt[:, off:off + wdt],
                                ident_sb[:],
                                kpos_sb[:, s * BW2 + side * BANDW + u:
                                           s * BW2 + side * BANDW + u + wdt],
                                start=False, stop=False,
                                skip_group_check=True)
                    # chunk max (negated) then exp+accum over killed logits
                    slot = s * NCH + ch
                    nc.vector.tensor_reduce(
                        negq_sb[:, slot:slot + 1], pt[:],
                        axis=mybir.AxisListType.X, op=mybir.AluOpType.max,
                        negate=True)
                    trash = trashp.tile([128, CHUNK], f32, tag="trash",
                                        name=f"trash{s}_{ch}")
                    nc.scalar.activation(
                        trash[:], pt[:],
                        mybir.ActivationFunctionType.Exp,
                        bias=negq_sb[:, slot:slot + 1],
                        accum_out=ssum_sb[:, slot:slot + 1])

                # band -> DRAM -> diagonal gather.  dram rows have stride
                # BW2+2=354; the per-row diagonal window [p, p+k] linearizes
                # to flat[p*355 + k], read back via a stride-355 view.
                dband = dbp.tile([128 * (BW2 + 3)], f32, tag="dband",
                                 name=f"dband{s}")
                flat = dband[:]
                wview = flat[0:128 * (BW2 + 2)].rearrange(
                    "(p u) -> p u", u=BW2 + 2)
                nc.sync.dma_start(wview[:, 0:BW2], band_sb[:])
                rview = flat[0:128 * (BW2 + 3)].rearrange(
                    "(p x) -> p x", x=BW2 + 3)
                nc.sync.dma_start(g_sb[:, s * GW: s * GW + KW], rview[:, 0:KW])
                nc.sync.dma_start(g_sb[:, s * GW + KW: s * GW + 2 * KW],
                                  rview[:, BANDW:BANDW + KW])

            # ---- end phase (batched over all stripes) --------------------
            negq3 = negq_sb[:].rearrange("p (s c) -> p s c", c=NCH)
            g3 = g_sb[:].rearrange("p (s k) -> p s k", k=GW)

            negbm = ep.tile([128, NSTR], f32, tag="negbm")
            nc.vector.tensor_reduce(negbm[:], g3, axis=mybir.AxisListType.X,
                                    op=mybir.AluOpType.max, negate=True)
            negm = ep.tile([128, NSTR], f32, tag="negm")
            nc.vector.tensor_reduce(negm[:], negq3, axis=mybir.AxisListType.X,
                                    op=mybir.AluOpType.min)
            nc.vector.tensor_tensor(negm[:], negm[:], negbm[:],
                                    op=mybir.AluOpType.min)

            def bcast(small, k, tag):
                # [128, NSTR] -> [128, NSTR, k] broadcast AP
                return small[:].unsqueeze(2).to_broadcast((128, NSTR, k))

            # S = sum_c Ssum_c * exp(negm - negq_c)
            efac = ep.tile([128, NSTR * NCH], f32, tag="efac")
            efac3 = efac[:].rearrange("p (s c) -> p s c", c=NCH)
            nc.vector.tensor_tensor(efac3, bcast(negm, NCH, "negmb"), negq3,
                                    op=mybir.AluOpType.subtract)
            nc.scalar.activation(efac[:], efac[:],
                                 mybir.ActivationFunctionType.Exp)
            nc.vector.tensor_tensor(efac[:], efac[:], ssum_sb[:],
                                    op=mybir.AluOpType.mult)
            S = ep.tile([128, NSTR], f32, tag="S")
            nc.vector.tensor_reduce(S[:], efac3, axis=mybir.AxisListType.X,
                                    op=mybir.AluOpType.add)

            # gm = g + negm ; eb = exp(gm)
            gm = ep.tile([128, NSTR * GW], f32, tag="gm")
            gm3 = gm[:].rearrange("p (s k) -> p s k", k=GW)
            nc.gpsimd.tensor_tensor(gm3, g3, bcast(negm, GW, "negmb2"),
                                    op=mybir.AluOpType.add)
            eb = ep.tile([128, NSTR * GW], f32, tag="eb")
            nc.scalar.activation(eb[:], gm[:], mybir.ActivationFunctionType.Exp)

            # ns_cam = S + sum (wid-wcam)*eb ; ns_id = S
            tmp = ep.tile([128, NSTR * GW], f32, tag="tmp")
            tmp3 = tmp[:].rearrange("p (s k) -> p s k", k=GW)
            nc.gpsimd.tensor_tensor(tmp[:], eb[:], wdiff_sb[:],
                                    op=mybir.AluOpType.mult)
            nscam = ep.tile([128, NSTR], f32, tag="nscam")
            nc.vector.tensor_reduce(nscam[:], tmp3, axis=mybir.AxisListType.X,
                                    op=mybir.AluOpType.add)
            nc.vector.tensor_tensor(nscam[:], nscam[:], S[:],
                                    op=mybir.AluOpType.add)

            rows = ep.tile([128, NSTR], f32, tag="rows")
            first = True
            for wsb, nssb, rnsb in ((wid_sb, S, rnid_sb),
                                    (wcam_sb, nscam, rncam_sb)):
                # t = ((eb - W) + 1) + ns  -- order preserves tiny ns exactly
                tb = ep.tile([128, NSTR * GW], f32, tag="tb",
                             name=f"tb{0 if first else 1}")
                tb3 = tb[:].rearrange("p (s k) -> p s k", k=GW)
                nc.gpsimd.tensor_tensor(tb[:], eb[:], wsb[:],
                                        op=mybir.AluOpType.subtract)
                nc.vector.scalar_tensor_tensor(
                    tb3, tb3, 1.0, bcast(nssb, GW, "nsb"),
                    op0=mybir.AluOpType.add, op1=mybir.AluOpType.add)
                nc.scalar.activation(tb[:], tb[:],
                                     mybir.ActivationFunctionType.Ln)
                nc.gpsimd.tensor_tensor(tb[:], tb[:], gm[:],
                                        op=mybir.AluOpType.subtract)
                nc.vector.tensor_tensor(tb[:], tb[:], wsb[:],
                                        op=mybir.AluOpType.mult)
                L = ep.tile([128, NSTR], f32, tag="L",
                            name=f"L{0 if first else 1}")
                nc.vector.tensor_reduce(L[:], tb3, axis=mybir.AxisListType.X,
                                        op=mybir.AluOpType.add)
                nc.vector.tensor_tensor(L[:], L[:], rnsb[:],
                                        op=mybir.AluOpType.mult)
                if first:
                    nc.vector.tensor_copy(rows[:], L[:])
                    first = False
                else:
                    nc.vector.scalar_tensor_tensor(
                        rows[:], L[:], 0.5, rows[:],
                        op0=mybir.AluOpType.mult, op1=mybir.AluOpType.add)

            rowsum = ep.tile([128, 1], f32, tag="rowsum")
            nc.vector.tensor_reduce(rowsum[:], rows[:],
                                    axis=mybir.AxisListType.X,
                                    op=mybir.AluOpType.add)
            nc.sync.dma_start(out_d.ap(), rowsum[:])

    nc.compile()
    _CACHE["nc"] = nc
    return nc


# --------------------------------------------------------------------------
# public entry point
# --------------------------------------------------------------------------

def kernel(features, label, camid):
    from concourse.bass_utils import run_bass_kernel_spmd

    features = np.asarray(features, np.float32)
    label_in = np.asarray(label)
    camid_in = np.asarray(camid)

    per_core, _ = _host_prep(features, label_in.astype(np.int64),
                             camid_in.astype(np.int64))
    nc = _build()
    in_maps = [{k: np.ascontiguousarray(v) for k, v in pc.items()}
               for pc in per_core]
    res = run_bass_kernel_spmd(nc, in_maps, core_ids=list(range(NCORE)))
    total = np.float32(0.0)
    outs = []
    for r in res.results:
        o = r["out"].astype(np.float32)
        outs.append(o)
        total += o.sum(dtype=np.float32)
    _CACHE["outs"] = outs
    loss = total / np.float32(N2)
    return np.asarray(loss, dtype=np.float32)


# revision 20
# speedup vs baseline: 1.3427x; 1.1666x over previous
"""TRN2 Bass kernel for nn_CamAwareSCLHead: cam-aware supervised contrastive loss.

Strategy (data-parallel over 8 NeuronCores, 1024 of 8192 logit rows each):
  host: sort samples by label so same-label positives form a narrow diagonal
        band; rotate each core's column space so its rows sit at cols [0,1024).
  device per row-stripe of 128 rows:
    PE   : fp16 matmul f_local @ f_full.T in [128,2048] chunks (fp32 accum),
           then identity-matmuls add -60000 on the diagonal and on all
           positive pairs (so the dense exp-sum covers negatives only).
    DVE  : per-chunk row max (negated).
    ACT  : exp(lg - chunkmax) with fused row-sum accumulation.
    band : raw logits of the +-24 diagonal band (positives live there) are
           copied out, bounced through DRAM to extract the per-row diagonal
           window, and all the log/mask math runs on tiny [128,768] tensors.
  The per-positive loss terms replicate the fp32 reference exactly, including
  its underflow behaviour (the fp32 reference of this problem is -inf).
"""
import sys

if '/opt/trn_rl_repo' not in sys.path:
    sys.path.insert(0, '/opt/trn_rl_repo')

import numpy as np

N = 4096
D = 256
N2 = 2 * N
TEMP = 0.1
NCORE = 8
RPC = N2 // NCORE          # rows per core
NSTR = RPC // 128          # row stripes per core
W = 24                     # half window of the diagonal band
KW = 2 * W                 # gathered slots per band side
BANDW = 128 + KW           # band buffer cols per side
BW2 = 2 * BANDW
BPAD = BANDW * 2 + 2       # dram row stride 354 => diagonal becomes p*354+k
CHUNK = 1024
NCH = N2 // CHUNK
KILL = np.float32(-60000.0)

_CACHE = {}
DEBUG_TAPS = False


# --------------------------------------------------------------------------
# host-side prep: sorting, rotation, window weights, kill masks
# --------------------------------------------------------------------------

def _host_prep(features, label, camid):
    f = np.concatenate([features[:, 0], features[:, 1]], axis=0).astype(np.float32)
    f = f * np.float32(np.sqrt(1.0 / TEMP))   # bake 1/TEMP into the matmul
    label = np.asarray(label).astype(np.int64)
    camid = np.asarray(camid).astype(np.int64)
    order = np.argsort(label, kind='stable')
    label_s = label[order]
    cam_s = camid[order]
    f_s = np.concatenate([f[:N][order], f[N:][order]], axis=0)

    # label block [start, end) per sorted sample
    change = np.flatnonzero(np.diff(label_s)) + 1
    bounds = np.concatenate([[0], change, [N]])
    blk_start = np.zeros(N, np.int64)
    blk_end = np.zeros(N, np.int64)
    for i in range(len(bounds) - 1):
        blk_start[bounds[i]:bounds[i + 1]] = bounds[i]
        blk_end[bounds[i]:bounds[i + 1]] = bounds[i + 1]
    assert (blk_end - blk_start).max() <= W, "label block exceeds band window"

    per_core = []
    for c in range(NCORE):
        rot = np.roll(f_s, -RPC * c, axis=0)
        fT = np.ascontiguousarray(rot.T).astype(np.float16)      # [D, N2]
        fT = np.ascontiguousarray(
            fT.reshape(2, 128, NCH, CHUNK).transpose(0, 2, 1, 3))  # [2,NCH,128,CHUNK]

        Wid = np.zeros((RPC, 2 * KW), np.float32)
        Wcam = np.zeros((RPC, 2 * KW), np.float32)
        Kpos = np.zeros((RPC, BW2), np.float32)
        p_arr = np.arange(RPC)
        g_arr = RPC * c + p_arr
        sig = g_arr % N
        for p in range(RPC):
            a, b = blk_start[sig[p]], blk_end[sig[p]]
            prow = p % 128
            for j in range(a, b):
                for half in (0, 1):
                    gcol = j + half * N
                    q = (gcol - RPC * c) % N2
                    dlt = (q - p + N2 // 2) % N2 - N2 // 2
                    if -W <= dlt < W:
                        k = dlt + W
                        u = prow + dlt + W
                    else:
                        d2 = dlt - N if dlt > 0 else dlt + N
                        assert -W <= d2 < W
                        k = KW + d2 + W
                        u = BANDW + prow + d2 + W
                    if gcol == (RPC * c + p):
                        continue                       # diag handled separately
                    Kpos[p, u] = KILL
                    Wid[p, k] = 1.0
                    if cam_s[j] == cam_s[sig[p]]:
                        Wcam[p, k] = 1.0

        def fold(a, width):        # [RPC, width] -> [128, NSTR*width]
            return np.ascontiguousarray(
                a.reshape(NSTR, 128, width).transpose(1, 0, 2).reshape(128, NSTR * width))

        rnid = (1.0 / Wid.sum(axis=1)).astype(np.float32)
        rncam = (1.0 / Wcam.sum(axis=1)).astype(np.float32)
        per_core.append({
            "ft": fT,
            "wid": fold(Wid, 2 * KW),
            "wcam": fold(Wcam, 2 * KW),
            "wdiff": fold(Wid - Wcam, 2 * KW),
            "kpos": fold(Kpos, BW2).astype(np.float16),
            "rnid": fold(rnid[:, None], 1),
            "rncam": fold(rncam[:, None], 1),
        })
    ident = np.eye(128, dtype=np.float16)
    kdiag = (np.float32(KILL) * np.eye(128)).astype(np.float16)
    for pc in per_core:
        pc["ident"] = ident
        pc["kdiag"] = kdiag
    return per_core, order


# --------------------------------------------------------------------------
# band piece geometry (static, same for all cores thanks to rotation)
# --------------------------------------------------------------------------

def _band_pieces(s, side):
    """Return [(chunk, off_in_chunk, width, buf_off)] for band `side` of stripe s.
    Splits at the N2 wrap; pieces are also split at 512 psum-bank boundaries
    by the caller when used for kill matmuls."""
    base = 128 * s - W + (N if side else 0)
    pieces = []
    u = 0
    while u < BANDW:
        col = (base + u) % N2
        # width until wrap or end
        wdt = min(BANDW - u, N2 - col)
        # split at chunk boundary
        ch = col // CHUNK
        wdt = min(wdt, (ch + 1) * CHUNK - col)
        pieces.append((ch, col - ch * CHUNK, wdt, u))
        u += wdt
    return pieces


def _split_bank(pieces):
    out = []
    for ch, off, wdt, u in pieces:
        while wdt > 0:
            w1 = min(wdt, 512 - (off % 512))
            out.append((ch, off, w1, u))
            off += w1; u += w1; wdt -= w1
    return out


# --------------------------------------------------------------------------
# bass kernel
# --------------------------------------------------------------------------

def _build():
    if "nc" in _CACHE:
        return _CACHE["nc"]
    import concourse.bass as bass
    import concourse.bacc as bacc
    import concourse.tile as tile
    from concourse import mybir

    f32 = mybir.dt.float32
    f16 = mybir.dt.float16

    nc = bacc.Bacc("TRN2", target_bir_lowering=False, debug=False,
                   num_devices=NCORE)

    ft_d = nc.dram_tensor("ft", [2, NCH, 128, CHUNK], f16, kind="ExternalInput")
    wid_d = nc.dram_tensor("wid", [128, NSTR * 2 * KW], f32, kind="ExternalInput")
    wcam_d = nc.dram_tensor("wcam", [128, NSTR * 2 * KW], f32, kind="ExternalInput")
    wdiff_d = nc.dram_tensor("wdiff", [128, NSTR * 2 * KW], f32, kind="ExternalInput")
    kpos_d = nc.dram_tensor("kpos", [128, NSTR * BW2], f16, kind="ExternalInput")
    rnid_d = nc.dram_tensor("rnid", [128, NSTR], f32, kind="ExternalInput")
    rncam_d = nc.dram_tensor("rncam", [128, NSTR], f32, kind="ExternalInput")
    ident_d = nc.dram_tensor("ident", [128, 128], f16, kind="ExternalInput")
    kdiag_d = nc.dram_tensor("kdiag", [128, 128], f16, kind="ExternalInput")
    out_d = nc.dram_tensor("out", [128, 1], f32, kind="ExternalOutput")

    GW = 2 * KW  # 96 gathered cols per stripe

    with tile.TileContext(nc) as tc:
        with (
            tc.tile_pool(name="persist", bufs=1) as pp,
            tc.tile_pool(name="bands", bufs=3) as bandp,
            tc.tile_pool(name="trash", bufs=3) as trashp,
            tc.tile_pool(name="psum", bufs=4, space="PSUM") as psp,
            tc.tile_pool(name="dramband", bufs=3, space="DRAM") as dbp,
            tc.tile_pool(name="endp", bufs=2) as ep,
        ):
            # ---- persistent inputs -> SBUF.  DMA order matters: the first
            # stripe needs ft[k][0] + masks before anything else.
            ft_sb = [[pp.tile([128, CHUNK], f16, tag=f"ft{k}_{ch}",
                              name=f"ftsb{k}_{ch}") for ch in range(NCH)]
                     for k in range(2)]
            kpos_sb = pp.tile([128, NSTR * BW2], f16, tag="kpos")
            ident_sb = pp.tile([128, 128], f16, tag="ident")
            kdiag_sb = pp.tile([128, 128], f16, tag="kdiag")
            wid_sb = pp.tile([128, NSTR * GW], f32, tag="wid")
            wcam_sb = pp.tile([128, NSTR * GW], f32, tag="wcam")
            wdiff_sb = pp.tile([128, NSTR * GW], f32, tag="wdiff")
            rnid_sb = pp.tile([128, NSTR], f32, tag="rnid")
            rncam_sb = pp.tile([128, NSTR], f32, tag="rncam")

            nc.sync.dma_start(ft_sb[0][0][:], ft_d[0, 0])
            nc.sync.dma_start(ft_sb[1][0][:], ft_d[1, 0])
            nc.sync.dma_start(ident_sb[:], ident_d.ap())
            nc.sync.dma_start(kdiag_sb[:], kdiag_d.ap())
            nc.sync.dma_start(kpos_sb[:], kpos_d.ap())
            for ch in range(1, NCH):
                for k in range(2):
                    nc.sync.dma_start(ft_sb[k][ch][:], ft_d[k, ch])
            for sb, dr in ((wid_sb, wid_d), (wcam_sb, wcam_d),
                           (wdiff_sb, wdiff_d), (rnid_sb, rnid_d),
                           (rncam_sb, rncam_d)):
                nc.sync.dma_start(sb[:], dr.ap())

            negq_sb = pp.tile([128, NSTR * NCH], f32, tag="negq")
            ssum_sb = pp.tile([128, NSTR * NCH], f32, tag="ssum")
            g_sb = pp.tile([128, NSTR * GW], f32, tag="g")
            rows_sb = pp.tile([128, NSTR * 2], f32, tag="rows")

            for s in range(NSTR):
                # ---------------- dense stripe pass -----------------------
                band_sb = bandp.tile([128, BW2], f32, tag="band",
                                     name=f"band{s}")
                piece_map = {}
                for side in (0, 1):
                    for pc in _band_pieces(s, side):
                        piece_map.setdefault(pc[0], []).append((side, pc))
                ncopy = 0
                for ch in range(NCH):
                    pt = psp.tile([128, CHUNK], f32, tag="ck", name=f"ck{s}_{ch}")
                    for k in range(2):
                        lhs = ft_sb[k][0][:, 128 * s:128 * (s + 1)]
                        for t in range(CHUNK // 512):
                            nc.tensor.matmul(
                                pt[:, t * 512:(t + 1) * 512],
                                lhs,
                                ft_sb[k][ch][:, t * 512:(t + 1) * 512],
                                start=(k == 0), stop=False,
                                skip_group_check=True,
                            )
                    if ch == (128 * s) // CHUNK:
                        off = 128 * s - ch * CHUNK
                        nc.tensor.matmul(
                            pt[:, off:off + 128],
                            kdiag_sb[:], ident_sb[:],
                            start=False, stop=False, skip_group_check=True)
                    for side, (pch, off, wdt, u) in piece_map.get(ch, []):
                        eng = nc.scalar if (ncopy % 2 == 0) else nc.vector
                        dst = band_sb[:, side * BANDW + u: side * BANDW + u + wdt]
                        if ncopy % 2 == 0:
                            nc.scalar.copy(dst, pt[:, off:off + wdt])
                        else:
                            nc.vector.tensor_copy(dst, pt[:, off:off + wdt])
                        ncopy += 1
                    for side, piece in piece_map.get(ch, []):
                        for pch, off, wdt, u in _split_bank([piece]):
                            nc.tensor.matmul(
                                pt[:, off:off + wdt],
                                ident_sb[:],
                                kpos_sb[:, s * BW2 + side * BANDW + u:
                                           s * BW2 + side * BANDW + u + wdt],
                                start=False, stop=False,
                                skip_group_check=True)
                    slot = s * NCH + ch
                    nc.vector.tensor_reduce(
                        negq_sb[:, slot:slot + 1], pt[:],
                        axis=mybir.AxisListType.X, op=mybir.AluOpType.max,
                        negate=True)
                    trash = trashp.tile([128, CHUNK], f32, tag="trash",
                                        name=f"trash{s}_{ch}")
                    nc.scalar.activation(
                        trash[:], pt[:],
                        mybir.ActivationFunctionType.Exp,
                        bias=negq_sb[:, slot:slot + 1],
                        accum_out=ssum_sb[:, slot:slot + 1])

                # band -> DRAM -> diagonal gather (stride-355 trick)
                dband = dbp.tile([128 * (BW2 + 3)], f32, tag="dband",
                                 name=f"dband{s}")
                flat = dband[:]
                wview = flat[0:128 * (BW2 + 2)].rearrange(
                    "(p u) -> p u", u=BW2 + 2)
                nc.sync.dma_start(wview[:, 0:BW2], band_sb[:])
                rview = flat[0:128 * (BW2 + 3)].rearrange(
                    "(p x) -> p x", x=BW2 + 3)
                gs = g_sb[:, s * GW: (s + 1) * GW]
                nc.sync.dma_start(g_sb[:, s * GW: s * GW + KW], rview[:, 0:KW])
                nc.sync.dma_start(g_sb[:, s * GW + KW: s * GW + 2 * KW],
                                  rview[:, BANDW:BANDW + KW])

                # ---------------- per-stripe tail (tiny ops) --------------
                nqs = negq_sb[:, s * NCH:(s + 1) * NCH]
                sss = ssum_sb[:, s * NCH:(s + 1) * NCH]

                negm = ep.tile([128, 1], f32, tag="negm", name=f"negm{s}")
                nc.vector.tensor_reduce(negm[:], gs, axis=mybir.AxisListType.X,
                                        op=mybir.AluOpType.max, negate=True)
                nqmin = ep.tile([128, 1], f32, tag="nqmin", name=f"nqmin{s}")
                nc.vector.tensor_reduce(nqmin[:], nqs,
                                        axis=mybir.AxisListType.X,
                                        op=mybir.AluOpType.min)
                nc.vector.tensor_tensor(negm[:], negm[:], nqmin[:],
                                        op=mybir.AluOpType.min)

                # S = sum_c Ssum_c * exp(negm - negq_c)
                efac = ep.tile([128, NCH], f32, tag="efac", name=f"efac{s}")
                nc.vector.tensor_scalar(
                    efac[:], nqs, negm[:], None,
                    op0=mybir.AluOpType.subtract)
                # efac = negq - negm; exp(scale=-1) gives exp(negm - negq)
                nc.scalar.activation(efac[:], efac[:],
                                     mybir.ActivationFunctionType.Exp,
                                     scale=-1.0)
                nc.vector.tensor_tensor(efac[:], efac[:], sss,
                                        op=mybir.AluOpType.mult)
                S = ep.tile([128, 1], f32, tag="S", name=f"S{s}")
                nc.vector.tensor_reduce(S[:], efac[:],
                                        axis=mybir.AxisListType.X,
                                        op=mybir.AluOpType.add)

                # gm = g + negm ; eb = exp(gm)
                gm = ep.tile([128, GW], f32, tag="gm", name=f"gm{s}")
                nc.vector.tensor_scalar_add(gm[:], gs, negm[:])
                eb = ep.tile([128, GW], f32, tag="eb", name=f"eb{s}")
                nc.scalar.activation(eb[:], gm[:],
                                     mybir.ActivationFunctionType.Exp)

                # ns_cam = S + sum (wid-wcam)*eb
                junk = ep.tile([128, GW], f32, tag="junk", name=f"junk{s}")
                nscam = ep.tile([128, 1], f32, tag="nscam", name=f"nscam{s}")
                nc.vector.tensor_tensor(junk[:], eb[:],
                                        wdiff_sb[:, s * GW:(s + 1) * GW],
                                        op=mybir.AluOpType.mult)
                nc.vector.tensor_reduce(nscam[:], junk[:],
                                        axis=mybir.AxisListType.X,
                                        op=mybir.AluOpType.add)
                nc.vector.tensor_tensor(nscam[:], nscam[:], S[:],
                                        op=mybir.AluOpType.add)

                for bi, (wsl, nssb, rnsl) in enumerate((
                        (wid_sb[:, s * GW:(s + 1) * GW], S,
                         rnid_sb[:, s:s + 1]),
                        (wcam_sb[:, s * GW:(s + 1) * GW], nscam,
                         rncam_sb[:, s:s + 1]))):
                    tb = ep.tile([128, GW], f32, tag=f"tb{bi}",
                                 name=f"tb{bi}_{s}")
                    nc.gpsimd.tensor_tensor(tb[:], eb[:], wsl,
                                            op=mybir.AluOpType.subtract)
                    # t = ((eb - W) + 1) + ns  (order preserves tiny ns)
                    nc.vector.scalar_tensor_tensor(
                        tb[:], tb[:], 1.0,
                        nssb[:].to_broadcast((128, GW)),
                        op0=mybir.AluOpType.add, op1=mybir.AluOpType.add)
                    nc.scalar.activation(tb[:], tb[:],
                                         mybir.ActivationFunctionType.Ln)
                    nc.gpsimd.tensor_tensor(tb[:], tb[:], gm[:],
                                            op=mybir.AluOpType.subtract)
                    L = ep.tile([128, 1], f32, tag=f"L{bi}",
                                name=f"L{bi}_{s}")
                    nc.vector.tensor_tensor(junk[:], tb[:], wsl,
                                            op=mybir.AluOpType.mult)
                    nc.vector.tensor_reduce(L[:], junk[:],
                                            axis=mybir.AxisListType.X,
                                            op=mybir.AluOpType.add)
                    # rows slot: 2s + bi  <- L * rn
                    nc.vector.tensor_tensor(
                        rows_sb[:, 2 * s + bi:2 * s + bi + 1], L[:], rnsl,
                        op=mybir.AluOpType.mult)

            # ---- final: rows_id + 0.5*rows_cam summed over stripes -------
            rows3 = rows_sb[:].rearrange("p (s b) -> p s b", b=2)
            rsum = ep.tile([128, 2], f32, tag="rsum")
            # sum over stripes for each branch: reduce middle axis -> use
            # strided views
            rid = rows_sb[:].rearrange("p (s b) -> p b s", b=2)
            nc.vector.tensor_reduce(rsum[:], rid, axis=mybir.AxisListType.X,
                                    op=mybir.AluOpType.add)
            rowsum = ep.tile([128, 1], f32, tag="rowsum")
            nc.vector.scalar_tensor_tensor(
                rowsum[:], rsum[:, 1:2], 0.5, rsum[:, 0:1],
                op0=mybir.AluOpType.mult, op1=mybir.AluOpType.add)
            nc.sync.dma_start(out_d.ap(), rowsum[:])

    nc.compile()
    _CACHE["nc"] = nc
    return nc


# --------------------------------------------------------------------------
# public entry point
# --------------------------------------------------------------------------

def kernel(features, label, camid):
    from concourse.bass_utils import run_bass_kernel_spmd

    features = np.asarray(features, np.float32)
    label_in = np.asarray(label)
    camid_in = np.asarray(camid)

    per_core, _ = _host_prep(features, label_in.astype(np.int64),
                             camid_in.astype(np.int64))
    nc = _build()
    in_maps = [{k: np.ascontiguousarray(v) for k, v in pc.items()}
               for pc in per_core]
    res = run_bass_kernel_spmd(nc, in_maps, core_ids=list(range(NCORE)))
    total = np.float32(0.0)
    outs = []
    for r in res.results:
        o = r["out"].astype(np.float32)
        outs.append(o)
        total += o.sum(dtype=np.float32)
    _CACHE["outs"] = outs
    loss = total / np.float32(N2)
    return np.asarray(loss, dtype=np.float32)
